# revision 1
# baseline (speedup 1.0000x reference)
"""Trainium2 Bass kernel for blur_pool2d -> per-(b,c) 25-bin histogram ->
cosine similarity -> scalar mean (nn_HIST_loss).

Sharding: data-parallel over batch, 4 batches (12 planes) per core x 8 cores.

Per-core device pipeline (per 512x512 plane, 24 planes = 12 x + 12 y):
  1. DMA plane into SBUF as 4x[128,512] f32
  2. Vertical Pascal conv (stride 2) = banded matmul on PE (f32, exact,
     zero-padding folded into band-clipped weights)  -> P in PSUM [256,512]
  3. Cast P -> bf16 SBUF (DVE+ACT halves)
  4. DMA xbar transpose (2-byte) -> PT bf16
  5. Horizontal Pascal conv = banded matmul on PE (bf16 in, f32 accum)
  6. Cast conv output -> v bf16 SBUF [128,512]
  7. 24 cumulative threshold counts (v >= j/25) with fused per-partition
     accumulation, split across DVE (is_ge) / ACT (Sign) / GPSIMD (is_ge)
  8. Cross-partition count reduction via ones-matmul at the end; DMA counts out
Host: finite-count diff -> histograms (exact integers up to bf16 binning,
      rel. effect ~1e-6), cosine in f64, mean.
"""

import numpy as np
import ml_dtypes

BINS = 25
N_CORES = 8
B_TOT, CH, H, W = 32, 3, 512, 512
PLANES_PER_CORE = (B_TOT // N_CORES) * CH  # 12
NPL = 2 * PLANES_PER_CORE                  # 24 (x planes then y planes)
TOTAL = (H // 2) * (W // 2)                # 65536 values per plane

# bin -> engine split (j = threshold index 1..24).  GPSIMD (Pool) does not
# support TensorScalarPtr, so only DVE + ACT count bins.
DVE_BINS = list(range(1, 15))    # 14
ACT_BINS = list(range(15, 25))   # 10 (Sign trick)
GPS_BINS = []
DVE_NC, ACT_NC, GPS_NC = len(DVE_BINS), len(ACT_BINS), len(GPS_BINS)
OUT_COLS = 512  # padded per-engine output row (NPL*DVE_NC=336 max, <=512)

_ROW = np.array([1., 6., 15., 20., 15., 6., 1.], dtype=np.float64) / 64.0


def _banded(n_in, n_out):
    """B[h,i] = row[h-2i+3] (zero-padding clipped)."""
    Bm = np.zeros((n_in, n_out), dtype=np.float32)
    for i in range(n_out):
        for b in range(7):
            h = 2 * i + b - 3
            if 0 <= h < n_in:
                Bm[h, i] = _ROW[b]
    return Bm


# (chunk, out_tile, start, stop) for both conv directions.  Each matmul uses
# the full [128,128] weight slice B[128c:128c+128, 128ot:128ot+128]; the band
# structure zero-pads everything outside the true range, and chunk/tile pairs
# whose slice is entirely zero are omitted.
_MM_PLAN = [
    (0, 0, True, False),
    (1, 0, False, False),
    (2, 0, False, True),
    (1, 1, True, False),
    (2, 1, False, False),
    (3, 1, False, True),
]

_CACHE = {}


def _build_module():
    import concourse.bass as bass
    import concourse.mybir as mybir
    import concourse.bacc as bacc
    import concourse.tile as tile

    f32 = mybir.dt.float32
    bf16 = mybir.dt.bfloat16

    nc = bacc.Bacc("TRN2", target_bir_lowering=False, debug=False,
                   num_devices=N_CORES)

    x_d = nc.dram_tensor("x", [PLANES_PER_CORE, H, W], f32, kind="ExternalInput")
    y_d = nc.dram_tensor("y", [PLANES_PER_CORE, H, W], f32, kind="ExternalInput")
    wb_d = nc.dram_tensor("wb", [H, H // 2], f32, kind="ExternalInput")
    wbh_d = nc.dram_tensor("wbh", [H, H // 2], bf16, kind="ExternalInput")
    cnt_d = nc.dram_tensor("cnt", [2, OUT_COLS], f32, kind="ExternalOutput")

    thr = [float(np.float32(j / BINS)) for j in range(BINS)]

    with tile.TileContext(nc) as tc:
        with tc.tile_pool(name="persist", bufs=1) as pp:
            # weights as 4 chunk tiles [128, 256]
            wv = pp.tile([128, 4, 256], f32, tag="wv")
            wh = pp.tile([128, 4, 256], bf16, tag="wh")
            nc.sync.dma_start(wv[:], wb_d.ap().rearrange("(c p) m -> p c m", p=128))
            nc.sync.dma_start(wh[:], wbh_d.ap().rearrange("(c p) m -> p c m", p=128))
            ones = pp.tile([128, 1], f32, tag="ones")
            nc.vector.memset(ones[:], 1.0)
            tneg = pp.tile([128, BINS], f32, tag="tneg")
            for j in range(1, BINS):
                nc.vector.memset(tneg[:, j:j + 1], -thr[j])
            scr_dve = pp.tile([128, 512], bf16, tag="scr_dve")
            scr_act = pp.tile([128, 512], bf16, tag="scr_act")
            acc_dve = pp.tile([128, NPL * DVE_NC], f32, tag="acc_dve")
            acc_act = pp.tile([128, NPL * ACT_NC], f32, tag="acc_act")

            with (
                tc.tile_pool(name="work", bufs=2) as wp,
                tc.tile_pool(name="mm", bufs=2, space=bass.MemorySpace.PSUM) as mp,
            ):
                for pl in range(NPL):
                    src = x_d if pl < PLANES_PER_CORE else y_d
                    idx = pl % PLANES_PER_CORE
                    tin = wp.tile([128, 4, 512], f32, tag="tin")
                    nc.sync.dma_start(
                        tin[:], src.ap()[idx].rearrange("(c p) w -> p c w", p=128))

                    p0 = mp.tile([128, 512], f32, tag="p0")
                    p1 = mp.tile([128, 512], f32, tag="p1")
                    pt_tiles = (p0, p1)
                    for c, ot, st, sp in _MM_PLAN:
                        nc.tensor.matmul(
                            pt_tiles[ot][:, :],
                            wv[:, c, 128 * ot:128 * ot + 128],
                            tin[:, c, :],
                            start=st, stop=sp,
                        )

                    pbf = wp.tile([128, 1024], bf16, tag="pbf")
                    nc.vector.tensor_copy(pbf[:, 0:512], p0[:])
                    nc.scalar.copy(pbf[:, 512:1024], p1[:])

                    ptr = wp.tile([128, 2, 4, 128], bf16, tag="ptr")
                    nc.sync.dma_start_transpose(ptr[:, 0, :, :], pbf[:, 0:512])
                    nc.sync.dma_start_transpose(ptr[:, 1, :, :], pbf[:, 512:1024])

                    o0 = mp.tile([128, 256], f32, tag="o0")
                    o1 = mp.tile([128, 256], f32, tag="o1")
                    ot_tiles = (o0, o1)
                    for c, ot, st, sp in _MM_PLAN:
                        nc.tensor.matmul(
                            ot_tiles[ot][:, :],
                            wh[:, c, 128 * ot:128 * ot + 128],
                            ptr[:, :, c, :],
                            start=st, stop=sp,
                        )

                    v = wp.tile([128, 512], bf16, tag="v")
                    nc.vector.tensor_copy(v[:, 0:256], o0[:])
                    nc.scalar.copy(v[:, 256:512], o1[:])

                    for k, j in enumerate(DVE_BINS):
                        c0 = pl * DVE_NC + k
                        nc.vector.tensor_scalar(
                            scr_dve[:], v[:], thr[j], None,
                            op0=mybir.AluOpType.is_ge,
                            op1=mybir.AluOpType.add,
                            accum_out=acc_dve[:, c0:c0 + 1])
                    for k, j in enumerate(ACT_BINS):
                        c0 = pl * ACT_NC + k
                        nc.scalar.activation(
                            scr_act[:], v[:],
                            mybir.ActivationFunctionType.Sign,
                            bias=tneg[:, j:j + 1],
                            accum_out=acc_act[:, c0:c0 + 1])

            with tc.tile_pool(name="cp", bufs=1,
                              space=bass.MemorySpace.PSUM) as cp:
                for r, acc, n in ((0, acc_dve, NPL * DVE_NC),
                                  (1, acc_act, NPL * ACT_NC)):
                    cps = cp.tile([1, OUT_COLS], f32, tag=f"c{r}")
                    csb = pp.tile([1, OUT_COLS], f32, tag=f"csb{r}")
                    nc.tensor.matmul(cps[:, 0:n], ones[:], acc[:, 0:n],
                                     start=True, stop=True)
                    nc.vector.tensor_copy(csb[:, 0:n], cps[:, 0:n])
                    nc.sync.dma_start(cnt_d.ap()[r:r + 1, 0:n], csb[:, 0:n])

    nc.compile()
    return nc


def _get_module():
    if "nc" not in _CACHE:
        _CACHE["nc"] = _build_module()
    return _CACHE["nc"]


def kernel(x: np.ndarray, y: np.ndarray) -> np.ndarray:
    res = run_raw(x, y)
    return _postprocess([r["cnt"] for r in res.results])


def run_raw(x, y, trace=False, **kw):
    from concourse.bass_utils import run_bass_kernel_spmd

    nc = _get_module()

    Bm = _banded(H, H // 2)
    Bbf = Bm.astype(ml_dtypes.bfloat16)
    bpc = B_TOT // N_CORES
    in_maps = []
    for i in range(N_CORES):
        in_maps.append({
            "x": np.ascontiguousarray(
                x[i * bpc:(i + 1) * bpc].reshape(PLANES_PER_CORE, H, W)),
            "y": np.ascontiguousarray(
                y[i * bpc:(i + 1) * bpc].reshape(PLANES_PER_CORE, H, W)),
            "wb": Bm,
            "wbh": Bbf,
        })

    return run_bass_kernel_spmd(nc, in_maps, core_ids=list(range(N_CORES)),
                                trace=trace, **kw)


def _postprocess(cnts):
    """cnts: per-core [3, OUT_COLS] f32 engine-count rows -> scalar mean."""
    cos_sum = 0.0
    n = 0
    for cnt in cnts:
        hx = np.zeros((PLANES_PER_CORE, BINS), dtype=np.float64)
        hy = np.zeros((PLANES_PER_CORE, BINS), dtype=np.float64)
        for pl in range(NPL):
            ge = np.zeros(BINS + 1, dtype=np.float64)  # ge[j] = #{v >= j/25}
            ge[0] = TOTAL
            for k, j in enumerate(DVE_BINS):
                ge[j] = cnt[0, pl * DVE_NC + k]
            for k, j in enumerate(ACT_BINS):
                ge[j] = (TOTAL + cnt[1, pl * ACT_NC + k]) / 2.0
            hist = ge[:-1] - ge[1:]
            if pl < PLANES_PER_CORE:
                hx[pl] = hist
            else:
                hy[pl - PLANES_PER_CORE] = hist
        for pl in range(PLANES_PER_CORE):
            a, b = hx[pl], hy[pl]
            na = max(np.linalg.norm(a), 1e-6 * TOTAL * 4)  # eps never binds
            nb = max(np.linalg.norm(b), 1e-6 * TOTAL * 4)
            cos_sum += float(np.dot(a, b) / (na * nb))
            n += 1
    return np.float32(cos_sum / n)

